# revision 14
# baseline (speedup 1.0000x reference)
"""Trainium2 Bass kernel for tanh-attention (nn_Attention_50362786513376).

reference:
  q = (x @ Wq.T) * dk^-0.5 ; k = x @ Wk.T ; v = x        (heads = 8, dk = 64)
  out = tanh(q k^T) v   per (batch, head),  merged back to [b, n, dim]

Sharding: 8 cores = 4 batches x 2 head-halves (4 heads per core).
Host pre-work (free, exact): transpose x[b] -> xT, slice v channels (fp8
e4m3 value v8 + e4m3 residual r8 for error feedback), slice + scale +
transpose weights. Device per core:
  Q^T = WqT.T @ xT, K^T = WkT.T @ xT     (f16)
  per head pair p, i-quarter iq, j256-tile j2:
    S^T[j,i] = K^T.T Q^T (row-packed pairs, f16, 2 tiles of [128,1024])
    tanh -> fp8e4 T tile [128, 2(kt), 1024(par,i)]:
      ScalarE activation for 5 of 8 j2's; custom-DVE degree-5 odd
      polynomial (clamped) for 3 of 8 -> splits the n^2 activation work
      across two engines.
    out^T[d,i] += DoubleRow-fp8 matmuls: (v8 + r8)^T @ T   (K=256/instr,
      4x column rate vs f16; r8 restores f16-level accuracy)
Host post-work: out[b,:,half] = outT.T
"""
import numpy as np

HEADS = 8
DK = 64
B = 4
N = 2048
DIM = 512
SCALE = DK ** (-0.5)
NCORES = 8
HALF = DIM // 2  # 256 channels per core (4 heads)

_built = None
_built_cfg = None
PROJ_DTYPE = "f16"   # x / weights / projection matmuls
ATTN_DTYPE = "f16"   # Q^T/K^T, qk matmuls
# tanh engine split: per j2 one kt tile goes to the Vector engine
# (custom-DVE polynomial) and one to ScalarE; the DVE tile's QK pair is
# emitted FIRST so the slower engine starts earlier and the AV matmul's
# operands finish just in time. 8 Act / 8 DVE tiles per iq.
R8_J2S = (1, 4, 6)   # j2 indices with the fp8 error-feedback matmul
                     # (also PE filler: keeps the tensor engine saturated
                     # so its clock never down-ramps)
TRACE = False
TRACE_KW = {}

# degree-5 odd minimax-ish polynomial for tanh (gaussian-weighted fit),
# clamped to [-1, 1]:  t = clip(x*(PC0 + y*(PC1 + y*PC2)), -1, 1), y = x^2
PC0, PC1, PC2 = 0.96814171, -0.2257031, 0.02721388


def _register_tanh_poly():
    """Register TANH_POLY5_ANT as a custom DVE op at a free opcode row."""
    import concourse.dve_ops as dve_ops
    from concourse.dve_ops import DveOp
    from concourse.dve_spec import (Spec, Src0, C0, C1, C2, Zero, One, sq,
                                    maxx, minn, lower)
    from concourse.dve_uop import DveOpSpec

    name = "TANH_POLY5_ANT"
    if name in dve_ops._SUB_OPCODE_FOR_NAME:
        return next(op for op in dve_ops.OPS if op.name == name)

    y = sq(Src0)
    body = minn(maxx(Src0 * (C0 + y * (C1 + y * C2)), Zero - One), One)

    def ref(in0, in1, s0, s1, imm2):
        yy = in0.astype(np.float32) ** 2
        return np.clip(in0 * (s0 + yy * (s1 + yy * imm2)), -1.0, 1.0).astype(
            np.float32)

    spec = Spec(body=body, reference=ref)
    row = max(dve_ops._SUB_OPCODE_FOR_NAME.values()) + 1
    assert row < 0x20
    shas = {}
    for ver in ("v3", "v4"):
        uops = lower(spec, ver=ver)
        tmp = DveOpSpec(name=name, opcode=row, uops=uops, rd1_en=False)
        shas[ver] = tmp.sha(ver)
    op = DveOp(name, spec, subdim=False, uops_sha=shas)
    dve_ops.OPS.append(op)
    dve_ops._SUB_OPCODE_FOR_NAME[name] = row
    dve_ops.CUSTOM_DVE_SPECS[name] = spec
    return op


def _build():
    from contextlib import ExitStack

    import concourse.tile as tile
    from concourse import bacc, mybir

    F32 = mybir.dt.float32
    F8 = mybir.dt.float8e4
    DT = {"f32r": mybir.dt.float32r, "f16": mybir.dt.float16,
          "bf16": mybir.dt.bfloat16}
    PROJ_DT = DT[PROJ_DTYPE]
    ATTN_DT = DT[ATTN_DTYPE]
    Tanh = mybir.ActivationFunctionType.Tanh
    DR = mybir.MatmulPerfMode.DoubleRow

    tanh_op = _register_tanh_poly()

    nc = bacc.Bacc("TRN2", target_bir_lowering=False, debug=False,
                   num_devices=NCORES)
    xT_ap = nc.dram_tensor("xT", [DIM, N], PROJ_DT, kind="ExternalInput").ap()
    xv_ap = nc.dram_tensor("xv", [N, HALF], F8, kind="ExternalInput").ap()
    xr_ap = nc.dram_tensor("xr", [N, HALF], F8, kind="ExternalInput").ap()
    wqT_ap = nc.dram_tensor("wqT", [DIM, HALF], PROJ_DT,
                            kind="ExternalInput").ap()
    wkT_ap = nc.dram_tensor("wkT", [DIM, HALF], PROJ_DT,
                            kind="ExternalInput").ap()
    outT_ap = nc.dram_tensor("outT", [HALF, N], F32, kind="ExternalOutput").ap()

    NJ2 = N // 256         # 8 j256-tiles

    with tile.TileContext(nc) as tc:
        with ExitStack() as ctx:
            const = ctx.enter_context(tc.tile_pool(name="const", bufs=1))
            qk_pool = ctx.enter_context(tc.tile_pool(name="qk", bufs=1))
            t_pool = ctx.enter_context(tc.tile_pool(name="tanh", bufs=6))
            stg_pool = ctx.enter_context(tc.tile_pool(name="stg", bufs=6))

            # ---- load inputs (xT on sync queue: projections chase its
            # chunks; weights on scalar queue; v8/r8 on gpsimd queue) ----
            xT_sb = const.tile([128, 4 * N], PROJ_DT)
            wq_sb = const.tile([128, 4 * HALF], PROJ_DT)
            wk_sb = const.tile([128, 4 * HALF], PROJ_DT)
            # first chunk split in two so the first proj matmul (needs only
            # cols 0:512) starts as early as possible
            nc.sync.dma_start(xT_sb[:, 0:512], xT_ap[0:128, 0:512])
            nc.sync.dma_start(xT_sb[:, 512:1024], xT_ap[0:128, 512:1024])
            for ct in range(1, 4):
                nc.sync.dma_start(xT_sb[:, ct * N:ct * N + 1024],
                                  xT_ap[ct * 128:(ct + 1) * 128, 0:1024])
            for w_sb, w_ap in ((wk_sb, wkT_ap), (wq_sb, wqT_ap)):
                for ct in range(4):
                    nc.scalar.dma_start(w_sb[:, ct * HALF:(ct + 1) * HALF],
                                        w_ap[ct * 128:(ct + 1) * 128, :])
            for ct in range(4):
                nc.scalar.dma_start(xT_sb[:, ct * N + 1024:ct * N + 2048],
                                    xT_ap[ct * 128:(ct + 1) * 128, 1024:2048])
            # v8/r8 [2048, 256] -> [128, 16(jt), 256] tiles, 16 DMAs each on
            # the gpsimd queue (keeps scalar/sync sequencers free)
            xv_sb = const.tile([128, 16, HALF], F8)
            xr_sb = const.tile([128, 16, HALF], F8)
            for j in range(16):
                nc.gpsimd.dma_start(xv_sb[:, j, :],
                                    xv_ap[j * 128:(j + 1) * 128, :])
            if R8_J2S:
                for j in range(16):
                    nc.gpsimd.dma_start(xr_sb[:, j, :],
                                        xr_ap[j * 128:(j + 1) * 128, :])

            # ---- projections + attention ----
            # PSUM: ps_S 3 bufs x [128,1024] (6 banks) + ps_acc 2 x [64,512]
            # (2 banks). Projection groups borrow ps_S/ps_acc slots.
            QT = [qk_pool.tile([128, N], ATTN_DT, tag=f"qt{p}", name=f"qt{p}")
                  for p in range(2)]
            KT = [qk_pool.tile([128, N], ATTN_DT, tag=f"kt{p}", name=f"kt{p}")
                  for p in range(2)]
            ps_S = ctx.enter_context(
                tc.tile_pool(name="ps_S", bufs=3, space="PSUM"))
            ps_acc = ctx.enter_context(
                tc.tile_pool(name="ps_acc", bufs=2, space="PSUM"))

            def proj_pair(dst, w_sb, p, t4_pair):
                ps2 = {t4_pair[0]: ps_S.tile([128, 512], F32, tag="S",
                                             name="proj_ps"),
                       t4_pair[1]: ps_acc.tile([128, 512], F32, tag="acc",
                                               name="proj_ps2")}
                for ct in range(4):
                    lhsT = w_sb[:, ct * HALF + p * 128:
                                ct * HALF + (p + 1) * 128]
                    for t4 in t4_pair:
                        rhs = xT_sb[:, ct * N + t4 * 512:
                                    ct * N + t4 * 512 + 512]
                        nc.tensor.matmul(ps2[t4][:], lhsT, rhs,
                                         start=(ct == 0), stop=(ct == 3))
                for t4 in t4_pair:
                    nc.scalar.copy(dst[p][:, t4 * 512:(t4 + 1) * 512],
                                   ps2[t4])

            for t4_pair in ((0, 1), (2, 3)):
                for p in range(2):
                    for dst, w_sb in ((KT, wk_sb), (QT, wq_sb)):
                        proj_pair(dst, w_sb, p, t4_pair)

            # ---- attention ----
            for p in range(2):
                for iq in range(4):          # i-quarter: i cols iq*512..+512
                    acc = [ps_acc.tile([64, 512], F32, tag="acc", name="acc")
                           for par in range(2)]
                    i0 = iq * 512
                    Ts = [None] * NJ2

                    def qk_tanh(j2):
                        T = t_pool.tile([128, 2, 1024], F8, tag="T", name="T")
                        # DVE tile (kt0) first: the slower engine gets a
                        # head start so both tanh tiles finish ~together
                        kt_dve = 0
                        for kt in (kt_dve, 1 - kt_dve):
                            j = 2 * j2 + kt
                            S = ps_S.tile([128, 1024], F32, tag="S", name="S")
                            nc.tensor.matmul(
                                S[:, 0:512],
                                KT[p][0:64, j * 128:(j + 1) * 128],
                                QT[p][0:64, i0:i0 + 512],
                                start=True, stop=True, tile_position=(0, 0))
                            nc.tensor.matmul(
                                S[:, 512:1024],
                                KT[p][64:128, j * 128:(j + 1) * 128],
                                QT[p][64:128, i0:i0 + 512],
                                start=True, stop=True, tile_position=(64, 0))
                            if kt == kt_dve:
                                nc.vector._custom_dve(
                                    tanh_op, out=T[:, kt, :], in0=S[:],
                                    s0=PC0, s1=PC1, imm2=PC2)
                            else:
                                nc.scalar.activation(T[:, kt, :], S[:], Tanh)
                        return T

                    def av(j2):
                        T = Ts[j2]
                        vs = (xv_sb, xr_sb) if j2 in R8_J2S else (xv_sb,)
                        for par in range(2):
                            lh = 2 * p + par
                            for vi, vsb in enumerate(vs):
                                nc.tensor.matmul(
                                    acc[par][:],
                                    vsb[:, 2 * j2:2 * j2 + 2,
                                        lh * 64:(lh + 1) * 64],
                                    T[:, :, par * 512:(par + 1) * 512],
                                    start=(j2 == 0 and vi == 0),
                                    stop=(j2 == NJ2 - 1 and vi == len(vs) - 1),
                                    perf_mode=DR)

                    # software pipeline: QK/tanh one j2 ahead of AV so the
                    # tensor engine never waits on the activation
                    Ts[0] = qk_tanh(0)
                    for j2 in range(1, NJ2):
                        Ts[j2] = qk_tanh(j2)
                        av(j2 - 1)
                    av(NJ2 - 1)

                    # drain copies split across both engines so neither
                    # stalls the next iq's tanh stream for long
                    for par in range(2):
                        lh = 2 * p + par
                        st = stg_pool.tile([64, 512], F32, tag="stg",
                                           name="stg")
                        nc.scalar.copy(st[:], acc[par][:])
                        nc.sync.dma_start(
                            outT_ap[lh * 64:(lh + 1) * 64,
                                    iq * 512:(iq + 1) * 512],
                            st[:])

    nc.compile()
    return nc


def _get_built():
    global _built, _built_cfg
    cfg = (PROJ_DTYPE, ATTN_DTYPE, R8_J2S)
    if _built is None or _built_cfg != cfg:
        _built = _build()
        _built_cfg = cfg
    return _built


def kernel(x, Wq, Wk):
    from concourse.bass_utils import run_bass_kernel_spmd

    x = np.asarray(x, dtype=np.float32)
    Wq = np.asarray(Wq, dtype=np.float32)
    Wk = np.asarray(Wk, dtype=np.float32)

    import ml_dtypes
    proj_np = np.float16 if PROJ_DTYPE == "f16" else np.float32
    E4 = ml_dtypes.float8_e4m3

    nc = _get_built()
    in_maps = []
    for c in range(NCORES):
        b, half = c // 2, c % 2
        sl = slice(half * HALF, (half + 1) * HALF)
        xv = x[b][:, sl]
        xv8 = xv.astype(E4)
        xr8 = (xv - xv8.astype(np.float32)).astype(E4)
        in_maps.append({
            "xT": np.ascontiguousarray(x[b].T).astype(proj_np),
            "xv": np.ascontiguousarray(xv8),
            "xr": np.ascontiguousarray(xr8),
            "wqT": np.ascontiguousarray((SCALE * Wq[sl, :]).T).astype(proj_np),
            "wkT": np.ascontiguousarray(Wk[sl, :].T).astype(proj_np),
        })
    try:
        res = run_bass_kernel_spmd(nc, in_maps, core_ids=list(range(NCORES)),
                                   trace=TRACE, **TRACE_KW)
    except Exception:
        # transient device wedge (NRT_EXEC_UNIT_UNRECOVERABLE) recovers on
        # retry; one attempt is enough in practice
        import time as _time
        _time.sleep(2.0)
        res = run_bass_kernel_spmd(nc, in_maps, core_ids=list(range(NCORES)),
                                   trace=TRACE, **TRACE_KW)
    out = np.empty((B, N, DIM), np.float32)
    for c in range(NCORES):
        b, half = c // 2, c % 2
        out[b, :, half * HALF:(half + 1) * HALF] = res.results[c]["outT"].T
    if TRACE:
        kernel.last_results = res
    return out


# revision 15
# speedup vs baseline: 1.0687x; 1.0687x over previous
"""Trainium2 Bass kernel for tanh-attention (nn_Attention_50362786513376).

reference:
  q = (x @ Wq.T) * dk^-0.5 ; k = x @ Wk.T ; v = x        (heads = 8, dk = 64)
  out = tanh(q k^T) v   per (batch, head),  merged back to [b, n, dim]

Sharding: 8 cores = 4 batches x 2 head-halves (4 heads per core).
Host pre-work (free, exact): transpose x[b] -> xT, slice v channels (fp8
e4m3 value v8 + e4m3 residual r8 for error feedback), slice + scale +
transpose weights. Device per core:
  Q^T = WqT.T @ xT, K^T = WkT.T @ xT     (f16)
  per head pair p, i-quarter iq, j256-tile j2:
    S^T[j,i] = K^T.T Q^T (row-packed pairs, f16, 2 tiles of [128,1024])
    tanh -> fp8e4 T tile [128, 2(kt), 1024(par,i)]:
      ScalarE activation for 5 of 8 j2's; custom-DVE degree-5 odd
      polynomial (clamped) for 3 of 8 -> splits the n^2 activation work
      across two engines.
    out^T[d,i] += DoubleRow-fp8 matmuls: (v8 + r8)^T @ T   (K=256/instr,
      4x column rate vs f16; r8 restores f16-level accuracy)
Host post-work: out[b,:,half] = outT.T
"""
import numpy as np

HEADS = 8
DK = 64
B = 4
N = 2048
DIM = 512
SCALE = DK ** (-0.5)
NCORES = 8
HALF = DIM // 2  # 256 channels per core (4 heads)

_built = None
_built_cfg = None
PROJ_DTYPE = "f16"   # x / weights / projection matmuls
ATTN_DTYPE = "f16"   # Q^T/K^T, qk matmuls
# tanh engine split: per j2 one kt tile goes to the Vector engine
# (custom-DVE polynomial) and one to ScalarE; the DVE tile's QK pair is
# emitted FIRST so the slower engine starts earlier and the AV matmul's
# operands finish just in time. 8 Act / 8 DVE tiles per iq.
R8_J2S = (2, 5)      # j2 indices with the fp8 error-feedback matmul
                     # (also PE filler: keeps the tensor engine saturated
                     # so its clock never down-ramps)
TRACE = False
TRACE_KW = {}

# degree-5 odd minimax-ish polynomial for tanh (gaussian-weighted fit),
# clamped to [-1, 1]:  t = clip(x*(PC0 + y*(PC1 + y*PC2)), -1, 1), y = x^2
PC0, PC1, PC2 = 0.96814171, -0.2257031, 0.02721388


def _register_tanh_poly():
    """Register TANH_POLY5_ANT as a custom DVE op at a free opcode row."""
    import concourse.dve_ops as dve_ops
    from concourse.dve_ops import DveOp
    from concourse.dve_spec import (Spec, Src0, C0, C1, C2, Zero, One, sq,
                                    maxx, minn, lower)
    from concourse.dve_uop import DveOpSpec

    name = "TANH_POLY5_ANT"
    if name in dve_ops._SUB_OPCODE_FOR_NAME:
        return next(op for op in dve_ops.OPS if op.name == name)

    y = sq(Src0)
    body = minn(maxx(Src0 * (C0 + y * (C1 + y * C2)), Zero - One), One)

    def ref(in0, in1, s0, s1, imm2):
        yy = in0.astype(np.float32) ** 2
        return np.clip(in0 * (s0 + yy * (s1 + yy * imm2)), -1.0, 1.0).astype(
            np.float32)

    spec = Spec(body=body, reference=ref)
    row = max(dve_ops._SUB_OPCODE_FOR_NAME.values()) + 1
    assert row < 0x20
    shas = {}
    for ver in ("v3", "v4"):
        uops = lower(spec, ver=ver)
        tmp = DveOpSpec(name=name, opcode=row, uops=uops, rd1_en=False)
        shas[ver] = tmp.sha(ver)
    op = DveOp(name, spec, subdim=False, uops_sha=shas)
    dve_ops.OPS.append(op)
    dve_ops._SUB_OPCODE_FOR_NAME[name] = row
    dve_ops.CUSTOM_DVE_SPECS[name] = spec
    return op


def _build():
    from contextlib import ExitStack

    import concourse.tile as tile
    from concourse import bacc, mybir

    F32 = mybir.dt.float32
    F8 = mybir.dt.float8e4
    DT = {"f32r": mybir.dt.float32r, "f16": mybir.dt.float16,
          "bf16": mybir.dt.bfloat16}
    PROJ_DT = DT[PROJ_DTYPE]
    ATTN_DT = DT[ATTN_DTYPE]
    Tanh = mybir.ActivationFunctionType.Tanh
    DR = mybir.MatmulPerfMode.DoubleRow

    tanh_op = _register_tanh_poly()

    nc = bacc.Bacc("TRN2", target_bir_lowering=False, debug=False,
                   num_devices=NCORES)
    xT_ap = nc.dram_tensor("xT", [DIM, N], PROJ_DT, kind="ExternalInput").ap()
    xv_ap = nc.dram_tensor("xv", [N, HALF], F8, kind="ExternalInput").ap()
    xr_ap = nc.dram_tensor("xr", [N, HALF], F8, kind="ExternalInput").ap()
    wqT_ap = nc.dram_tensor("wqT", [DIM, HALF], PROJ_DT,
                            kind="ExternalInput").ap()
    wkT_ap = nc.dram_tensor("wkT", [DIM, HALF], PROJ_DT,
                            kind="ExternalInput").ap()
    outT_ap = nc.dram_tensor("outT", [HALF, N], F32, kind="ExternalOutput").ap()

    NJ2 = N // 256         # 8 j256-tiles

    with tile.TileContext(nc) as tc:
        with ExitStack() as ctx:
            const = ctx.enter_context(tc.tile_pool(name="const", bufs=1))
            qk_pool = ctx.enter_context(tc.tile_pool(name="qk", bufs=1))
            t_pool = ctx.enter_context(tc.tile_pool(name="tanh", bufs=6))
            stg_pool = ctx.enter_context(tc.tile_pool(name="stg", bufs=6))

            # ---- load inputs (xT on sync queue: projections chase its
            # chunks; weights on scalar queue; v8/r8 on gpsimd queue) ----
            xT_sb = const.tile([128, 4 * N], PROJ_DT)
            wq_sb = const.tile([128, 4 * HALF], PROJ_DT)
            wk_sb = const.tile([128, 4 * HALF], PROJ_DT)
            # first chunk split in two so the first proj matmul (needs only
            # cols 0:512) starts as early as possible
            nc.sync.dma_start(xT_sb[:, 0:512], xT_ap[0:128, 0:512])
            nc.sync.dma_start(xT_sb[:, 512:1024], xT_ap[0:128, 512:1024])
            for ct in range(1, 4):
                nc.sync.dma_start(xT_sb[:, ct * N:ct * N + 1024],
                                  xT_ap[ct * 128:(ct + 1) * 128, 0:1024])
            for w_sb, w_ap in ((wk_sb, wkT_ap), (wq_sb, wqT_ap)):
                for ct in range(4):
                    nc.scalar.dma_start(w_sb[:, ct * HALF:(ct + 1) * HALF],
                                        w_ap[ct * 128:(ct + 1) * 128, :])
            for ct in range(4):
                nc.scalar.dma_start(xT_sb[:, ct * N + 1024:ct * N + 2048],
                                    xT_ap[ct * 128:(ct + 1) * 128, 1024:2048])
            # v8/r8 [2048, 256] -> [128, 16(jt), 256] tiles, 16 DMAs each on
            # the gpsimd queue (keeps scalar/sync sequencers free)
            xv_sb = const.tile([128, 16, HALF], F8)
            xr_sb = const.tile([128, 16, HALF], F8)
            for j in range(16):
                nc.gpsimd.dma_start(xv_sb[:, j, :],
                                    xv_ap[j * 128:(j + 1) * 128, :])
            if R8_J2S:
                for j in range(16):
                    nc.gpsimd.dma_start(xr_sb[:, j, :],
                                        xr_ap[j * 128:(j + 1) * 128, :])

            # ---- projections + attention ----
            # PSUM: ps_S 3 bufs x [128,1024] (6 banks) + ps_acc 2 x [64,512]
            # (2 banks). Projection groups borrow ps_S/ps_acc slots.
            QT = [qk_pool.tile([128, N], ATTN_DT, tag=f"qt{p}", name=f"qt{p}")
                  for p in range(2)]
            KT = [qk_pool.tile([128, N], ATTN_DT, tag=f"kt{p}", name=f"kt{p}")
                  for p in range(2)]
            ps_S = ctx.enter_context(
                tc.tile_pool(name="ps_S", bufs=3, space="PSUM"))
            ps_acc = ctx.enter_context(
                tc.tile_pool(name="ps_acc", bufs=2, space="PSUM"))

            def proj_pair(dst, w_sb, p, t4_pair):
                ps2 = {t4_pair[0]: ps_S.tile([128, 512], F32, tag="S",
                                             name="proj_ps"),
                       t4_pair[1]: ps_acc.tile([128, 512], F32, tag="acc",
                                               name="proj_ps2")}
                for ct in range(4):
                    lhsT = w_sb[:, ct * HALF + p * 128:
                                ct * HALF + (p + 1) * 128]
                    for t4 in t4_pair:
                        rhs = xT_sb[:, ct * N + t4 * 512:
                                    ct * N + t4 * 512 + 512]
                        nc.tensor.matmul(ps2[t4][:], lhsT, rhs,
                                         start=(ct == 0), stop=(ct == 3))
                for t4 in t4_pair:
                    nc.vector.tensor_copy(dst[p][:, t4 * 512:(t4 + 1) * 512],
                                          ps2[t4])

            for t4_pair in ((0, 1), (2, 3)):
                for p in range(2):
                    for dst, w_sb in ((KT, wk_sb), (QT, wq_sb)):
                        proj_pair(dst, w_sb, p, t4_pair)

            # ---- attention ----
            for p in range(2):
                for iq in range(4):          # i-quarter: i cols iq*512..+512
                    acc = [ps_acc.tile([64, 512], F32, tag="acc", name="acc")
                           for par in range(2)]
                    i0 = iq * 512
                    Ts = [None] * NJ2

                    def qk_tanh(j2):
                        T = t_pool.tile([128, 2, 1024], F8, tag="T", name="T")
                        # DVE tile first (slower engine head start);
                        # j2 == 0 runs both tiles on ScalarE
                        kt_dve = -1 if j2 == 0 else 1
                        for kt in ((1, 0) if j2 else (0, 1)):
                            j = 2 * j2 + kt
                            S = ps_S.tile([128, 1024], F32, tag="S", name="S")
                            nc.tensor.matmul(
                                S[:, 0:512],
                                KT[p][0:64, j * 128:(j + 1) * 128],
                                QT[p][0:64, i0:i0 + 512],
                                start=True, stop=True, tile_position=(0, 0))
                            nc.tensor.matmul(
                                S[:, 512:1024],
                                KT[p][64:128, j * 128:(j + 1) * 128],
                                QT[p][64:128, i0:i0 + 512],
                                start=True, stop=True, tile_position=(64, 0))
                            if kt == kt_dve:  # noqa: SIM114
                                nc.vector._custom_dve(
                                    tanh_op, out=T[:, kt, :], in0=S[:],
                                    s0=PC0, s1=PC1, imm2=PC2)
                            else:
                                nc.scalar.activation(T[:, kt, :], S[:], Tanh)
                        return T

                    def av(j2):
                        T = Ts[j2]
                        vs = (xv_sb, xr_sb) if j2 in R8_J2S else (xv_sb,)
                        for par in range(2):
                            lh = 2 * p + par
                            for vi, vsb in enumerate(vs):
                                nc.tensor.matmul(
                                    acc[par][:],
                                    vsb[:, 2 * j2:2 * j2 + 2,
                                        lh * 64:(lh + 1) * 64],
                                    T[:, :, par * 512:(par + 1) * 512],
                                    start=(j2 == 0 and vi == 0),
                                    stop=(j2 == NJ2 - 1 and vi == len(vs) - 1),
                                    perf_mode=DR)

                    # software pipeline: QK/tanh two j2 ahead of AV so the
                    # tensor engine never waits on the activation
                    Ts[0] = qk_tanh(0)
                    Ts[1] = qk_tanh(1)
                    for j2 in range(2, NJ2):
                        Ts[j2] = qk_tanh(j2)
                        av(j2 - 2)
                    av(NJ2 - 2)
                    av(NJ2 - 1)

                    # drain copies split across both engines so neither
                    # stalls the next iq's tanh stream for long
                    for par in range(2):
                        lh = 2 * p + par
                        st = stg_pool.tile([64, 512], F32, tag="stg",
                                           name="stg")
                        nc.vector.tensor_copy(st[:], acc[par][:])
                        nc.sync.dma_start(
                            outT_ap[lh * 64:(lh + 1) * 64,
                                    iq * 512:(iq + 1) * 512],
                            st[:])

    nc.compile()
    return nc


def _get_built():
    global _built, _built_cfg
    cfg = (PROJ_DTYPE, ATTN_DTYPE, R8_J2S)
    if _built is None or _built_cfg != cfg:
        _built = _build()
        _built_cfg = cfg
    return _built


def kernel(x, Wq, Wk):
    from concourse.bass_utils import run_bass_kernel_spmd

    x = np.asarray(x, dtype=np.float32)
    Wq = np.asarray(Wq, dtype=np.float32)
    Wk = np.asarray(Wk, dtype=np.float32)

    import ml_dtypes
    proj_np = np.float16 if PROJ_DTYPE == "f16" else np.float32
    E4 = ml_dtypes.float8_e4m3

    nc = _get_built()
    in_maps = []
    for c in range(NCORES):
        b, half = c // 2, c % 2
        sl = slice(half * HALF, (half + 1) * HALF)
        xv = x[b][:, sl]
        xv8 = xv.astype(E4)
        xr8 = (xv - xv8.astype(np.float32)).astype(E4)
        in_maps.append({
            "xT": np.ascontiguousarray(x[b].T).astype(proj_np),
            "xv": np.ascontiguousarray(xv8),
            "xr": np.ascontiguousarray(xr8),
            "wqT": np.ascontiguousarray((SCALE * Wq[sl, :]).T).astype(proj_np),
            "wkT": np.ascontiguousarray(Wk[sl, :].T).astype(proj_np),
        })
    try:
        res = run_bass_kernel_spmd(nc, in_maps, core_ids=list(range(NCORES)),
                                   trace=TRACE, **TRACE_KW)
    except Exception:
        # transient device wedge (NRT_EXEC_UNIT_UNRECOVERABLE) recovers on
        # retry; one attempt is enough in practice
        import time as _time
        _time.sleep(2.0)
        res = run_bass_kernel_spmd(nc, in_maps, core_ids=list(range(NCORES)),
                                   trace=TRACE, **TRACE_KW)
    out = np.empty((B, N, DIM), np.float32)
    for c in range(NCORES):
        b, half = c // 2, c % 2
        out[b, :, half * HALF:(half + 1) * HALF] = res.results[c]["outT"].T
    if TRACE:
        kernel.last_results = res
    return out


# revision 17
# speedup vs baseline: 1.0735x; 1.0045x over previous
"""Trainium2 Bass kernel for tanh-attention (nn_Attention_50362786513376).

reference:
  q = (x @ Wq.T) * dk^-0.5 ; k = x @ Wk.T ; v = x        (heads = 8, dk = 64)
  out = tanh(q k^T) v   per (batch, head),  merged back to [b, n, dim]

Sharding: 8 cores = 4 batches x 2 head-halves (4 heads per core).
Host pre-work (free, exact): transpose x[b] -> xT, slice v channels (fp8
e4m3 value v8 + e4m3 residual r8 for error feedback), slice + scale +
transpose weights. Device per core:
  Q^T = WqT.T @ xT, K^T = WkT.T @ xT     (f16)
  per head pair p, i-quarter iq, j256-tile j2:
    S^T[j,i] = K^T.T Q^T (row-packed pairs, f16, 2 tiles of [128,1024])
    tanh -> fp8e4 T tile [128, 2(kt), 1024(par,i)]:
      ScalarE activation for 5 of 8 j2's; custom-DVE degree-5 odd
      polynomial (clamped) for 3 of 8 -> splits the n^2 activation work
      across two engines.
    out^T[d,i] += DoubleRow-fp8 matmuls: (v8 + r8)^T @ T   (K=256/instr,
      4x column rate vs f16; r8 restores f16-level accuracy)
Host post-work: out[b,:,half] = outT.T
"""
import numpy as np

HEADS = 8
DK = 64
B = 4
N = 2048
DIM = 512
SCALE = DK ** (-0.5)
NCORES = 8
HALF = DIM // 2  # 256 channels per core (4 heads)

_built = None
_built_cfg = None
PROJ_DTYPE = "f16"   # x / weights / projection matmuls
ATTN_DTYPE = "f16"   # Q^T/K^T, qk matmuls
# tanh engine split: per j2 one kt tile goes to the Vector engine
# (custom-DVE polynomial) and one to ScalarE; the DVE tile's QK pair is
# emitted FIRST so the slower engine starts earlier and the AV matmul's
# operands finish just in time. 8 Act / 8 DVE tiles per iq.
R8_J2S = (2, 5)      # j2 indices with the fp8 error-feedback matmul
                     # (also PE filler: keeps the tensor engine saturated
                     # so its clock never down-ramps)
TRACE = False
TRACE_KW = {}

# degree-5 odd minimax-ish polynomial for tanh (gaussian-weighted fit),
# clamped to [-1, 1]:  t = clip(x*(PC0 + y*(PC1 + y*PC2)), -1, 1), y = x^2
PC0, PC1, PC2 = 0.96814171, -0.2257031, 0.02721388


def _register_tanh_poly():
    """Register TANH_POLY5_ANT as a custom DVE op at a free opcode row."""
    import concourse.dve_ops as dve_ops
    from concourse.dve_ops import DveOp
    from concourse.dve_spec import (Spec, Src0, C0, C1, C2, Zero, One, sq,
                                    maxx, minn, lower)
    from concourse.dve_uop import DveOpSpec

    name = "TANH_POLY5_ANT"
    if name in dve_ops._SUB_OPCODE_FOR_NAME:
        return next(op for op in dve_ops.OPS if op.name == name)

    y = sq(Src0)
    body = minn(maxx(Src0 * (C0 + y * (C1 + y * C2)), Zero - One), One)

    def ref(in0, in1, s0, s1, imm2):
        yy = in0.astype(np.float32) ** 2
        return np.clip(in0 * (s0 + yy * (s1 + yy * imm2)), -1.0, 1.0).astype(
            np.float32)

    spec = Spec(body=body, reference=ref)
    row = max(dve_ops._SUB_OPCODE_FOR_NAME.values()) + 1
    assert row < 0x20
    shas = {}
    for ver in ("v3", "v4"):
        uops = lower(spec, ver=ver)
        tmp = DveOpSpec(name=name, opcode=row, uops=uops, rd1_en=False)
        shas[ver] = tmp.sha(ver)
    op = DveOp(name, spec, subdim=False, uops_sha=shas)
    dve_ops.OPS.append(op)
    dve_ops._SUB_OPCODE_FOR_NAME[name] = row
    dve_ops.CUSTOM_DVE_SPECS[name] = spec
    return op


def _build():
    from contextlib import ExitStack

    import concourse.tile as tile
    from concourse import bacc, mybir

    F32 = mybir.dt.float32
    F8 = mybir.dt.float8e4
    DT = {"f32r": mybir.dt.float32r, "f16": mybir.dt.float16,
          "bf16": mybir.dt.bfloat16}
    PROJ_DT = DT[PROJ_DTYPE]
    ATTN_DT = DT[ATTN_DTYPE]
    Tanh = mybir.ActivationFunctionType.Tanh
    DR = mybir.MatmulPerfMode.DoubleRow

    tanh_op = _register_tanh_poly()

    nc = bacc.Bacc("TRN2", target_bir_lowering=False, debug=False,
                   num_devices=NCORES)
    xT_ap = nc.dram_tensor("xT", [DIM, N], PROJ_DT, kind="ExternalInput").ap()
    xv_ap = nc.dram_tensor("xv", [N, HALF], F8, kind="ExternalInput").ap()
    xr_ap = nc.dram_tensor("xr", [N, HALF], F8, kind="ExternalInput").ap()
    wqT_ap = nc.dram_tensor("wqT", [DIM, HALF], PROJ_DT,
                            kind="ExternalInput").ap()
    wkT_ap = nc.dram_tensor("wkT", [DIM, HALF], PROJ_DT,
                            kind="ExternalInput").ap()
    outT_ap = nc.dram_tensor("outT", [HALF, N], F32, kind="ExternalOutput").ap()

    NJ2 = N // 256         # 8 j256-tiles

    with tile.TileContext(nc) as tc:
        with ExitStack() as ctx:
            const = ctx.enter_context(tc.tile_pool(name="const", bufs=1))
            qk_pool = ctx.enter_context(tc.tile_pool(name="qk", bufs=1))
            t_pool = ctx.enter_context(tc.tile_pool(name="tanh", bufs=6))
            stg_pool = ctx.enter_context(tc.tile_pool(name="stg", bufs=6))

            # ---- load inputs (xT on sync queue: projections chase its
            # chunks; weights on scalar queue; v8/r8 on gpsimd queue) ----
            xT_sb = const.tile([128, 4 * N], PROJ_DT)
            wq_sb = const.tile([128, 4 * HALF], PROJ_DT)
            wk_sb = const.tile([128, 4 * HALF], PROJ_DT)
            # spread input DMAs over all four queues so the projection
            # operands land in parallel: sync ct0/ct1 heads, vector ct2/ct3
            # heads, scalar weights, gpsimd xT tails then v8/r8
            nc.sync.dma_start(xT_sb[:, 0:512], xT_ap[0:128, 0:512])
            nc.sync.dma_start(xT_sb[:, 512:1024], xT_ap[0:128, 512:1024])
            nc.sync.dma_start(xT_sb[:, N:N + 1024],
                              xT_ap[128:256, 0:1024])
            for ct in (2, 3):
                nc.gpsimd.dma_start(xT_sb[:, ct * N:ct * N + 1024],
                                    xT_ap[ct * 128:(ct + 1) * 128, 0:1024])
            for w_sb, w_ap in ((wk_sb, wkT_ap), (wq_sb, wqT_ap)):
                for ct in range(4):
                    nc.scalar.dma_start(w_sb[:, ct * HALF:(ct + 1) * HALF],
                                        w_ap[ct * 128:(ct + 1) * 128, :])
            for ct in range(4):
                nc.gpsimd.dma_start(xT_sb[:, ct * N + 1024:ct * N + 2048],
                                    xT_ap[ct * 128:(ct + 1) * 128, 1024:2048])
            # v8/r8 [2048, 256] -> [128, 16(jt), 256] tiles, 16 DMAs each
            xv_sb = const.tile([128, 16, HALF], F8)
            xr_sb = const.tile([128, 16, HALF], F8)
            for j in range(16):
                nc.gpsimd.dma_start(xv_sb[:, j, :],
                                    xv_ap[j * 128:(j + 1) * 128, :])
            if R8_J2S:
                for j in range(16):
                    nc.gpsimd.dma_start(xr_sb[:, j, :],
                                        xr_ap[j * 128:(j + 1) * 128, :])

            # ---- projections + attention ----
            # PSUM: ps_S 3 bufs x [128,1024] (6 banks) + ps_acc 2 x [64,512]
            # (2 banks). Projection groups borrow ps_S/ps_acc slots.
            QT = [qk_pool.tile([128, N], ATTN_DT, tag=f"qt{p}", name=f"qt{p}")
                  for p in range(2)]
            KT = [qk_pool.tile([128, N], ATTN_DT, tag=f"kt{p}", name=f"kt{p}")
                  for p in range(2)]
            ps_S = ctx.enter_context(
                tc.tile_pool(name="ps_S", bufs=3, space="PSUM"))
            ps_acc = ctx.enter_context(
                tc.tile_pool(name="ps_acc", bufs=2, space="PSUM"))

            def proj_pair(dst, w_sb, p, t4_pair):
                ps2 = {t4_pair[0]: ps_S.tile([128, 512], F32, tag="S",
                                             name="proj_ps"),
                       t4_pair[1]: ps_acc.tile([128, 512], F32, tag="acc",
                                               name="proj_ps2")}
                for ct in range(4):
                    lhsT = w_sb[:, ct * HALF + p * 128:
                                ct * HALF + (p + 1) * 128]
                    for t4 in t4_pair:
                        rhs = xT_sb[:, ct * N + t4 * 512:
                                    ct * N + t4 * 512 + 512]
                        nc.tensor.matmul(ps2[t4][:], lhsT, rhs,
                                         start=(ct == 0), stop=(ct == 3))
                for t4 in t4_pair:
                    nc.vector.tensor_copy(dst[p][:, t4 * 512:(t4 + 1) * 512],
                                          ps2[t4])

            for t4_pair in ((0, 1), (2, 3)):
                for p in range(2):
                    for dst, w_sb in ((KT, wk_sb), (QT, wq_sb)):
                        proj_pair(dst, w_sb, p, t4_pair)

            # ---- attention ----
            for p in range(2):
                for iq in range(4):          # i-quarter: i cols iq*512..+512
                    acc = [ps_acc.tile([64, 512], F32, tag="acc", name="acc")
                           for par in range(2)]
                    i0 = iq * 512
                    Ts = [None] * NJ2

                    def qk_tanh(j2):
                        T = t_pool.tile([128, 2, 1024], F8, tag="T", name="T")
                        # DVE tile first (slower engine head start);
                        # j2 == 0 runs both tiles on ScalarE
                        kt_dve = -1 if j2 == 0 else 1
                        for kt in ((1, 0) if j2 else (0, 1)):
                            j = 2 * j2 + kt
                            S = ps_S.tile([128, 1024], F32, tag="S", name="S")
                            nc.tensor.matmul(
                                S[:, 0:512],
                                KT[p][0:64, j * 128:(j + 1) * 128],
                                QT[p][0:64, i0:i0 + 512],
                                start=True, stop=True, tile_position=(0, 0))
                            nc.tensor.matmul(
                                S[:, 512:1024],
                                KT[p][64:128, j * 128:(j + 1) * 128],
                                QT[p][64:128, i0:i0 + 512],
                                start=True, stop=True, tile_position=(64, 0))
                            if kt == kt_dve:  # noqa: SIM114
                                nc.vector._custom_dve(
                                    tanh_op, out=T[:, kt, :], in0=S[:],
                                    s0=PC0, s1=PC1, imm2=PC2)
                            else:
                                nc.scalar.activation(T[:, kt, :], S[:], Tanh)
                        return T

                    def av(j2):
                        T = Ts[j2]
                        vs = (xv_sb, xr_sb) if j2 in R8_J2S else (xv_sb,)
                        for par in range(2):
                            lh = 2 * p + par
                            for vi, vsb in enumerate(vs):
                                nc.tensor.matmul(
                                    acc[par][:],
                                    vsb[:, 2 * j2:2 * j2 + 2,
                                        lh * 64:(lh + 1) * 64],
                                    T[:, :, par * 512:(par + 1) * 512],
                                    start=(j2 == 0 and vi == 0),
                                    stop=(j2 == NJ2 - 1 and vi == len(vs) - 1),
                                    perf_mode=DR)

                    # software pipeline: QK/tanh two j2 ahead of AV so the
                    # tensor engine never waits on the activation
                    Ts[0] = qk_tanh(0)
                    Ts[1] = qk_tanh(1)
                    for j2 in range(2, NJ2):
                        Ts[j2] = qk_tanh(j2)
                        av(j2 - 2)
                    av(NJ2 - 2)
                    av(NJ2 - 1)

                    # drain copies split across both engines so neither
                    # stalls the next iq's tanh stream for long
                    for par in range(2):
                        lh = 2 * p + par
                        st = stg_pool.tile([64, 512], F32, tag="stg",
                                           name="stg")
                        nc.vector.tensor_copy(st[:], acc[par][:])
                        nc.sync.dma_start(
                            outT_ap[lh * 64:(lh + 1) * 64,
                                    iq * 512:(iq + 1) * 512],
                            st[:])

    nc.compile()
    return nc


def _get_built():
    global _built, _built_cfg
    cfg = (PROJ_DTYPE, ATTN_DTYPE, R8_J2S)
    if _built is None or _built_cfg != cfg:
        _built = _build()
        _built_cfg = cfg
    return _built


def kernel(x, Wq, Wk):
    from concourse.bass_utils import run_bass_kernel_spmd

    x = np.asarray(x, dtype=np.float32)
    Wq = np.asarray(Wq, dtype=np.float32)
    Wk = np.asarray(Wk, dtype=np.float32)

    import ml_dtypes
    proj_np = np.float16 if PROJ_DTYPE == "f16" else np.float32
    E4 = ml_dtypes.float8_e4m3

    nc = _get_built()
    in_maps = []
    for c in range(NCORES):
        b, half = c // 2, c % 2
        sl = slice(half * HALF, (half + 1) * HALF)
        xv = x[b][:, sl]
        xv8 = xv.astype(E4)
        xr8 = (xv - xv8.astype(np.float32)).astype(E4)
        in_maps.append({
            "xT": np.ascontiguousarray(x[b].T).astype(proj_np),
            "xv": np.ascontiguousarray(xv8),
            "xr": np.ascontiguousarray(xr8),
            "wqT": np.ascontiguousarray((SCALE * Wq[sl, :]).T).astype(proj_np),
            "wkT": np.ascontiguousarray(Wk[sl, :].T).astype(proj_np),
        })
    try:
        res = run_bass_kernel_spmd(nc, in_maps, core_ids=list(range(NCORES)),
                                   trace=TRACE, **TRACE_KW)
    except Exception:
        # transient device wedge (NRT_EXEC_UNIT_UNRECOVERABLE) recovers on
        # retry; one attempt is enough in practice
        import time as _time
        _time.sleep(2.0)
        res = run_bass_kernel_spmd(nc, in_maps, core_ids=list(range(NCORES)),
                                   trace=TRACE, **TRACE_KW)
    out = np.empty((B, N, DIM), np.float32)
    for c in range(NCORES):
        b, half = c // 2, c % 2
        out[b, :, half * HALF:(half + 1) * HALF] = res.results[c]["outT"].T
    if TRACE:
        kernel.last_results = res
    return out


# revision 18
# speedup vs baseline: 1.0737x; 1.0001x over previous
"""Trainium2 Bass kernel for tanh-attention (nn_Attention_50362786513376).

reference:
  q = (x @ Wq.T) * dk^-0.5 ; k = x @ Wk.T ; v = x        (heads = 8, dk = 64)
  out = tanh(q k^T) v   per (batch, head),  merged back to [b, n, dim]

Sharding: 8 cores = 4 batches x 2 head-halves (4 heads per core).
Host pre-work (free, exact): transpose x[b] -> xT, slice v channels (fp8
e4m3 value v8 + e4m3 residual r8 for error feedback), slice + scale +
transpose weights. Device per core:
  Q^T = WqT.T @ xT, K^T = WkT.T @ xT     (f16)
  per head pair p, i-quarter iq, j256-tile j2:
    S^T[j,i] = K^T.T Q^T (tile_position row-packed pairs stream
      ~concurrently on the PE, f16, 2 tiles of [128,1024])
    tanh -> fp8e4 T tile [128, 2(kt), 1024(par,i)]: the n^2 activation
      work is split across two engines per j2 -- one kt tile on ScalarE
      (hardware tanh), one on the Vector engine via a custom-DVE
      degree-5 odd clamped polynomial (emitted first: slower engine gets
      a head start). AV runs two j2 behind QK/tanh so the tensor engine
      never waits on the activations.
    out^T[d,i] += DoubleRow-fp8 matmuls: v8^T @ T (K=256/instr, 2x
      column rate vs f16; e4m3 residual r8 matmuls on 2 of 8 j2 claw
      back accuracy and keep the PE saturated)
Host post-work: out[b,:,half] = outT.T
"""
import numpy as np

HEADS = 8
DK = 64
B = 4
N = 2048
DIM = 512
SCALE = DK ** (-0.5)
NCORES = 8
HALF = DIM // 2  # 256 channels per core (4 heads)

_built = None
_built_cfg = None
PROJ_DTYPE = "f16"   # x / weights / projection matmuls
ATTN_DTYPE = "f16"   # Q^T/K^T, qk matmuls
# tanh engine split: per j2 one kt tile goes to the Vector engine
# (custom-DVE polynomial) and one to ScalarE; the DVE tile's QK pair is
# emitted FIRST so the slower engine starts earlier and the AV matmul's
# operands finish just in time. 8 Act / 8 DVE tiles per iq.
R8_J2S = (2, 5)      # j2 indices with the fp8 error-feedback matmul
                     # (also PE filler: keeps the tensor engine saturated
                     # so its clock never down-ramps)
TRACE = False
TRACE_KW = {}

# degree-5 odd minimax-ish polynomial for tanh (gaussian-weighted fit),
# clamped to [-1, 1]:  t = clip(x*(PC0 + y*(PC1 + y*PC2)), -1, 1), y = x^2
PC0, PC1, PC2 = 0.96814171, -0.2257031, 0.02721388


def _register_tanh_poly():
    """Register TANH_POLY5_ANT as a custom DVE op at a free opcode row."""
    import concourse.dve_ops as dve_ops
    from concourse.dve_ops import DveOp
    from concourse.dve_spec import (Spec, Src0, C0, C1, C2, Zero, One, sq,
                                    maxx, minn, lower)
    from concourse.dve_uop import DveOpSpec

    name = "TANH_POLY5_ANT"
    if name in dve_ops._SUB_OPCODE_FOR_NAME:
        return next(op for op in dve_ops.OPS if op.name == name)

    y = sq(Src0)
    body = minn(maxx(Src0 * (C0 + y * (C1 + y * C2)), Zero - One), One)

    def ref(in0, in1, s0, s1, imm2):
        yy = in0.astype(np.float32) ** 2
        return np.clip(in0 * (s0 + yy * (s1 + yy * imm2)), -1.0, 1.0).astype(
            np.float32)

    spec = Spec(body=body, reference=ref)
    row = max(dve_ops._SUB_OPCODE_FOR_NAME.values()) + 1
    assert row < 0x20
    shas = {}
    for ver in ("v3", "v4"):
        uops = lower(spec, ver=ver)
        tmp = DveOpSpec(name=name, opcode=row, uops=uops, rd1_en=False)
        shas[ver] = tmp.sha(ver)
    op = DveOp(name, spec, subdim=False, uops_sha=shas)
    dve_ops.OPS.append(op)
    dve_ops._SUB_OPCODE_FOR_NAME[name] = row
    dve_ops.CUSTOM_DVE_SPECS[name] = spec
    return op


def _build():
    from contextlib import ExitStack

    import concourse.tile as tile
    from concourse import bacc, mybir

    F32 = mybir.dt.float32
    F8 = mybir.dt.float8e4
    DT = {"f32r": mybir.dt.float32r, "f16": mybir.dt.float16,
          "bf16": mybir.dt.bfloat16}
    PROJ_DT = DT[PROJ_DTYPE]
    ATTN_DT = DT[ATTN_DTYPE]
    Tanh = mybir.ActivationFunctionType.Tanh
    DR = mybir.MatmulPerfMode.DoubleRow

    tanh_op = _register_tanh_poly()

    nc = bacc.Bacc("TRN2", target_bir_lowering=False, debug=False,
                   num_devices=NCORES)
    xT_ap = nc.dram_tensor("xT", [DIM, N], PROJ_DT, kind="ExternalInput").ap()
    xv_ap = nc.dram_tensor("xv", [N, HALF], F8, kind="ExternalInput").ap()
    xr_ap = nc.dram_tensor("xr", [N, HALF], F8, kind="ExternalInput").ap()
    wqT_ap = nc.dram_tensor("wqT", [DIM, HALF], PROJ_DT,
                            kind="ExternalInput").ap()
    wkT_ap = nc.dram_tensor("wkT", [DIM, HALF], PROJ_DT,
                            kind="ExternalInput").ap()
    outT_ap = nc.dram_tensor("outT", [HALF, N], F32, kind="ExternalOutput").ap()

    NJ2 = N // 256         # 8 j256-tiles

    with tile.TileContext(nc) as tc:
        with ExitStack() as ctx:
            const = ctx.enter_context(tc.tile_pool(name="const", bufs=1))
            qk_pool = ctx.enter_context(tc.tile_pool(name="qk", bufs=1))
            t_pool = ctx.enter_context(tc.tile_pool(name="tanh", bufs=6))
            stg_pool = ctx.enter_context(tc.tile_pool(name="stg", bufs=6))

            # ---- load inputs (xT on sync queue: projections chase its
            # chunks; weights on scalar queue; v8/r8 on gpsimd queue) ----
            xT_sb = const.tile([128, 4 * N], PROJ_DT)
            wq_sb = const.tile([128, 4 * HALF], PROJ_DT)
            wk_sb = const.tile([128, 4 * HALF], PROJ_DT)
            # spread input DMAs over all four queues so the projection
            # operands land in parallel: sync ct0/ct1 heads, vector ct2/ct3
            # heads, scalar weights, gpsimd xT tails then v8/r8
            nc.sync.dma_start(xT_sb[:, 0:512], xT_ap[0:128, 0:512])
            nc.sync.dma_start(xT_sb[:, 512:1024], xT_ap[0:128, 512:1024])
            nc.sync.dma_start(xT_sb[:, N:N + 1024],
                              xT_ap[128:256, 0:1024])
            for ct in (2, 3):
                nc.gpsimd.dma_start(xT_sb[:, ct * N:ct * N + 1024],
                                    xT_ap[ct * 128:(ct + 1) * 128, 0:1024])
            for w_sb, w_ap in ((wk_sb, wkT_ap), (wq_sb, wqT_ap)):
                for ct in range(4):
                    nc.scalar.dma_start(w_sb[:, ct * HALF:(ct + 1) * HALF],
                                        w_ap[ct * 128:(ct + 1) * 128, :])
            for ct in range(4):
                nc.gpsimd.dma_start(xT_sb[:, ct * N + 1024:ct * N + 2048],
                                    xT_ap[ct * 128:(ct + 1) * 128, 1024:2048])
            # v8/r8 [2048, 256] -> [128, 16(jt), 256] tiles, 16 DMAs each
            xv_sb = const.tile([128, 16, HALF], F8)
            xr_sb = const.tile([128, 16, HALF], F8)
            for j in range(16):
                nc.gpsimd.dma_start(xv_sb[:, j, :],
                                    xv_ap[j * 128:(j + 1) * 128, :])
            if R8_J2S:
                for j in range(16):
                    nc.gpsimd.dma_start(xr_sb[:, j, :],
                                        xr_ap[j * 128:(j + 1) * 128, :])

            # ---- projections + attention ----
            # PSUM: ps_S 3 bufs x [128,1024] (6 banks) + ps_acc 2 x [64,512]
            # (2 banks). Projection groups borrow ps_S/ps_acc slots.
            QT = [qk_pool.tile([128, N], ATTN_DT, tag=f"qt{p}", name=f"qt{p}")
                  for p in range(2)]
            KT = [qk_pool.tile([128, N], ATTN_DT, tag=f"kt{p}", name=f"kt{p}")
                  for p in range(2)]
            ps_S = ctx.enter_context(
                tc.tile_pool(name="ps_S", bufs=3, space="PSUM"))
            ps_acc = ctx.enter_context(
                tc.tile_pool(name="ps_acc", bufs=2, space="PSUM"))

            def proj_pair(dst, w_sb, p, t4_pair):
                ps2 = {t4_pair[0]: ps_S.tile([128, 512], F32, tag="S",
                                             name="proj_ps"),
                       t4_pair[1]: ps_acc.tile([128, 512], F32, tag="acc",
                                               name="proj_ps2")}
                for ct in range(4):
                    lhsT = w_sb[:, ct * HALF + p * 128:
                                ct * HALF + (p + 1) * 128]
                    for t4 in t4_pair:
                        rhs = xT_sb[:, ct * N + t4 * 512:
                                    ct * N + t4 * 512 + 512]
                        nc.tensor.matmul(ps2[t4][:], lhsT, rhs,
                                         start=(ct == 0), stop=(ct == 3))
                for t4 in t4_pair:
                    nc.vector.tensor_copy(dst[p][:, t4 * 512:(t4 + 1) * 512],
                                          ps2[t4])

            for t4_pair in ((0, 1), (2, 3)):
                for p in range(2):
                    for dst, w_sb in ((KT, wk_sb), (QT, wq_sb)):
                        proj_pair(dst, w_sb, p, t4_pair)

            # ---- attention ----
            for p in range(2):
                for iq in range(4):          # i-quarter: i cols iq*512..+512
                    acc = [ps_acc.tile([64, 512], F32, tag="acc", name="acc")
                           for par in range(2)]
                    i0 = iq * 512
                    Ts = [None] * NJ2

                    def qk_tanh(j2):
                        T = t_pool.tile([128, 2, 1024], F8, tag="T", name="T")
                        # DVE tile first (slower engine head start);
                        # j2 == 0 runs both tiles on ScalarE
                        kt_dve = -1 if j2 == 0 else 1
                        for kt in ((1, 0) if j2 else (0, 1)):
                            j = 2 * j2 + kt
                            S = ps_S.tile([128, 1024], F32, tag="S", name="S")
                            nc.tensor.matmul(
                                S[:, 0:512],
                                KT[p][0:64, j * 128:(j + 1) * 128],
                                QT[p][0:64, i0:i0 + 512],
                                start=True, stop=True, tile_position=(0, 0))
                            nc.tensor.matmul(
                                S[:, 512:1024],
                                KT[p][64:128, j * 128:(j + 1) * 128],
                                QT[p][64:128, i0:i0 + 512],
                                start=True, stop=True, tile_position=(64, 0))
                            if kt == kt_dve:  # noqa: SIM114
                                nc.vector._custom_dve(
                                    tanh_op, out=T[:, kt, :], in0=S[:],
                                    s0=PC0, s1=PC1, imm2=PC2)
                            else:
                                nc.scalar.activation(T[:, kt, :], S[:], Tanh)
                        return T

                    def av(j2):
                        T = Ts[j2]
                        vs = (xv_sb, xr_sb) if j2 in R8_J2S else (xv_sb,)
                        for par in range(2):
                            lh = 2 * p + par
                            for vi, vsb in enumerate(vs):
                                nc.tensor.matmul(
                                    acc[par][:],
                                    vsb[:, 2 * j2:2 * j2 + 2,
                                        lh * 64:(lh + 1) * 64],
                                    T[:, :, par * 512:(par + 1) * 512],
                                    start=(j2 == 0 and vi == 0),
                                    stop=(j2 == NJ2 - 1 and vi == len(vs) - 1),
                                    perf_mode=DR)

                    # software pipeline: QK/tanh two j2 ahead of AV so the
                    # tensor engine never waits on the activation
                    Ts[0] = qk_tanh(0)
                    Ts[1] = qk_tanh(1)
                    for j2 in range(2, NJ2):
                        Ts[j2] = qk_tanh(j2)
                        av(j2 - 2)
                    av(NJ2 - 2)
                    av(NJ2 - 1)

                    # drain copies split across both engines so neither
                    # stalls the next iq's tanh stream for long
                    for par in range(2):
                        lh = 2 * p + par
                        st = stg_pool.tile([64, 512], F32, tag="stg",
                                           name="stg")
                        nc.vector.tensor_copy(st[:], acc[par][:])
                        nc.sync.dma_start(
                            outT_ap[lh * 64:(lh + 1) * 64,
                                    iq * 512:(iq + 1) * 512],
                            st[:])

    nc.compile()
    return nc


def _get_built():
    global _built, _built_cfg
    cfg = (PROJ_DTYPE, ATTN_DTYPE, R8_J2S)
    if _built is None or _built_cfg != cfg:
        _built = _build()
        _built_cfg = cfg
    return _built


def kernel(x, Wq, Wk):
    from concourse.bass_utils import run_bass_kernel_spmd

    x = np.asarray(x, dtype=np.float32)
    Wq = np.asarray(Wq, dtype=np.float32)
    Wk = np.asarray(Wk, dtype=np.float32)

    import ml_dtypes
    proj_np = np.float16 if PROJ_DTYPE == "f16" else np.float32
    E4 = ml_dtypes.float8_e4m3

    nc = _get_built()
    in_maps = []
    for c in range(NCORES):
        b, half = c // 2, c % 2
        sl = slice(half * HALF, (half + 1) * HALF)
        xv = x[b][:, sl]
        xv8 = xv.astype(E4)
        xr8 = (xv - xv8.astype(np.float32)).astype(E4)
        in_maps.append({
            "xT": np.ascontiguousarray(x[b].T).astype(proj_np),
            "xv": np.ascontiguousarray(xv8),
            "xr": np.ascontiguousarray(xr8),
            "wqT": np.ascontiguousarray((SCALE * Wq[sl, :]).T).astype(proj_np),
            "wkT": np.ascontiguousarray(Wk[sl, :].T).astype(proj_np),
        })
    try:
        res = run_bass_kernel_spmd(nc, in_maps, core_ids=list(range(NCORES)),
                                   trace=TRACE, **TRACE_KW)
    except Exception:
        # transient device wedge (NRT_EXEC_UNIT_UNRECOVERABLE) recovers on
        # retry; one attempt is enough in practice
        import time as _time
        _time.sleep(2.0)
        res = run_bass_kernel_spmd(nc, in_maps, core_ids=list(range(NCORES)),
                                   trace=TRACE, **TRACE_KW)
    out = np.empty((B, N, DIM), np.float32)
    for c in range(NCORES):
        b, half = c // 2, c % 2
        out[b, :, half * HALF:(half + 1) * HALF] = res.results[c]["outT"].T
    if TRACE:
        kernel.last_results = res
    return out
